# revision 19
# baseline (speedup 1.0000x reference)
"""GQA kernel for Trainium2, 8 NeuronCores.

Problem: B=2, T=2048, D=2048, 16 query heads / 2 KV heads, d_head=128, causal.

Sharding: core c -> batch b = c//4, head-quarter q = c%4 (query heads
4q..4q+3, kv head q//2). Each core computes its 4 heads' attention and a
partial output projection (its Wo rows); host sums the 4 partials per batch
and adds bo.

Host marshalling: weights and x are pre-cast to bf16 and pre-tiled so each
logical input lands with one multi-dim DMA per priority chunk:
  xt [128, 16, 2048] = x[b].T tiled as (p, kb, t)
  wq [128, 16, 512], wk/wv [128, 16, 128]  (p, kb, cols)
  wo [128, 4, 2048]                        (p=dh, h, n)
Inputs stream on FOUR queues (sync/gpsimd/vector/scalar) in approximate
consumption order (KB blocks consumed 0..15): the early-engine queues
(vector/scalar) only carry startup chunks, before their first compute.

PE warm-up: the HAM clock gate keeps the PE at 1.2 GHz until ~3.4us of
sustained activity; dummy matmuls bridge the initial DMA wait so real
matmuls start at 2.4 GHz.

On-core dataflow (bf16 matmuls, fp32 PSUM):
per 512-wide t-slice j: K/V projections into one [128,2,512] PSUM pair
tile (ACT epilogues w/ bias); V PE-transposed to natural layout; Q heads
pairwise in PSUM pair tiles.  Attention per head h over tk blocks: S^T
computed per tk-block PAIR into a [128,2,512] PSUM pair tile; ONE fused
exp per pair (ACT reads both banks in one ACTIVATE); causal diagonal
blocks column-trimmed with all four S matmuls issued before their
exp/mask/PV chains (PE never sits in the chain); 128x128 affine_select
masks on gpsimd only for the true diagonal; PV accumulated into otps;
softmax denominator: bf16 pair-tree + fp32 racc on DVE, bf16 cast, one
ones-matmul partition reduction per (h,j); normalize (reciprocal+mul) on
DVE.  Per-head normalize is deferred into the next head's stream.
Output projection for slice j is cut into 4-matmul units used as PE
filler inside slice j+1's attention rounds; a few units are RESERVED to
cover the final head's normalize chain; slice-3 outproj uses freed
PSUM pair tiles and one [128,2048] DMA per row-block.
"""

import numpy as np
import ml_dtypes
from contextlib import ExitStack

import concourse.bass as bass
from concourse import bacc
import concourse.mybir as mybir
import concourse.tile as tile
from concourse.bass_utils import run_bass_kernel_spmd
from concourse.masks import make_identity
from concourse import bass_isa

F32 = mybir.dt.float32
F32R = mybir.dt.float32r
BF16 = mybir.dt.bfloat16

D = 2048
T = 2048
DH = 128
B = 2
HPC = 4            # query heads per core
NCORES = 8
SCALE = 1.0 / float(np.sqrt(128.0))
N_WARM = 9         # PE warm-up dummy matmuls (512-col)
ROWSUM_GPSIMD = True  # denominator partition-reduce on gpsimd vs PE matmul

_CACHE = {}


def _build_nc():
    nc = bacc.Bacc("TRN2", target_bir_lowering=False, debug=False,
                   num_devices=NCORES)

    xt = nc.dram_tensor("xt", [128, 4, 16, 512], BF16, kind="ExternalInput")
    wq = nc.dram_tensor("wq", [128, HPC, 16, DH], BF16, kind="ExternalInput")
    wk = nc.dram_tensor("wk", [128, 16, DH], BF16, kind="ExternalInput")
    wv = nc.dram_tensor("wv", [128, 16, DH], BF16, kind="ExternalInput")
    wo = nc.dram_tensor("wo", [128, HPC, D], BF16, kind="ExternalInput")
    bqm = nc.dram_tensor("bqm", [DH, HPC], F32, kind="ExternalInput")
    bkm = nc.dram_tensor("bkm", [DH, 1], F32, kind="ExternalInput")
    bvm = nc.dram_tensor("bvm", [DH, 1], F32, kind="ExternalInput")
    part = nc.dram_tensor("part", [T, D], BF16, kind="ExternalOutput")

    with ExitStack() as ctx:
        tc = ctx.enter_context(tile.TileContext(nc))
        persist = ctx.enter_context(tc.tile_pool(name="persist", bufs=1))
        work = ctx.enter_context(tc.tile_pool(name="work", bufs=2))
        psum = ctx.enter_context(tc.tile_pool(name="psum", bufs=2, space="PSUM"))

        # ---- constants ----
        ones32 = persist.tile([128, 128], BF16, tag="ones32", name="ones32")
        nc.vector.memset(ones32, 1.0)
        scr512 = persist.tile([128, 512], BF16, tag="scr512", name="scr512")
        nc.vector.memset(scr512, 0.0)
        ident = persist.tile([128, 128], BF16, tag="ident", name="ident")
        make_identity(nc, ident)
        bq_sb = persist.tile([DH, HPC], F32, tag="bq", name="bq_sb")
        bk_sb = persist.tile([DH, 1], F32, tag="bk", name="bk_sb")
        bv_sb = persist.tile([DH, 1], F32, tag="bv", name="bv_sb")

        # ---- persistent input tiles ----
        x_all = persist.tile([128, 4, 16, 512], BF16, tag="x_all", name="x_all")
        wq_sb = persist.tile([128, HPC, 16, DH], BF16, tag="wq", name="wq_sb")
        wk_sb = persist.tile([128, 16, DH], BF16, tag="wk", name="wk_sb")
        wv_sb = persist.tile([128, 16, DH], BF16, tag="wv", name="wv_sb")
        wo_sb = persist.tile([128, HPC, D], BF16, tag="wo", name="wo_sb")

        # ---- input DMAs ----
        # Single fast queue (gpsimd) in exact consumption order; all DMAs
        # are contiguous on both sides (x is slice-major, wq head-major).
        # Only wo (needed ~70us later) rides the slow sync queue.
        nc.gpsimd.dma_start(out=bq_sb, in_=bqm[:, :])
        nc.gpsimd.dma_start(out=bk_sb, in_=bkm[:, :])
        nc.gpsimd.dma_start(out=bv_sb, in_=bvm[:, :])
        nc.gpsimd.dma_start(out=wk_sb, in_=wk[:, :, :])
        nc.gpsimd.dma_start(out=wv_sb, in_=wv[:, :, :])
        for duo in range(6):
            a = slice(2 * duo, 2 * duo + 2)
            nc.gpsimd.dma_start(out=x_all[:, 0, a, :], in_=xt[:, 0, a, :])
        nc.gpsimd.dma_start(out=wq_sb[:, 0], in_=wq[:, 0])
        nc.gpsimd.dma_start(out=x_all[:, 0, 12:14, :], in_=xt[:, 0, 12:14, :])
        nc.gpsimd.dma_start(out=x_all[:, 0, 14:16, :], in_=xt[:, 0, 14:16, :])
        for h in range(1, 4):
            nc.gpsimd.dma_start(out=wq_sb[:, h], in_=wq[:, h])
        for js in range(1, 4):
            nc.gpsimd.dma_start(out=x_all[:, js, 0:8, :], in_=xt[:, js, 0:8, :])
            nc.gpsimd.dma_start(out=x_all[:, js, 8:16, :],
                                in_=xt[:, js, 8:16, :])
        nc.sync.dma_start(out=wo_sb[:, 0:2, :], in_=wo[:, 0:2, :])
        nc.sync.dma_start(out=wo_sb[:, 2:4, :], in_=wo[:, 2:4, :])

        # warm the ACT exp table-set (~2.7us) during the initial DMA wait
        warm = persist.tile([128, 1], F32, tag="warm", name="warm")
        nc.scalar.activation(out=warm, in_=ident[:, 0:1],
                             func=mybir.ActivationFunctionType.Exp)

        # ---- PE warm-up dummies: keep the HAM clock gate open ----
        wps = psum.tile([128, 2, 512], F32, tag="sps", bufs=2, name="wps")
        for i in range(N_WARM):
            nc.tensor.matmul(out=wps[:, i % 2, :], lhsT=ones32, rhs=scr512,
                             start=True, stop=True)

        # ---- persistent activations ----
        kT = persist.tile([128, T], BF16, tag="kT", name="kT")
        v_sb = [persist.tile([128, DH], BF16, tag=f"v{t}", name=f"v{t}")
                for t in range(16)]
        # per-slice q and o (o double-buffered: outproj(j) runs during j+1)
        qT4 = [persist.tile([128, 4, 512], BF16, tag=f"qT{h}", name=f"qT{h}")
               for h in range(HPC)]
        oT = [[persist.tile([128, 512], BF16, tag=f"oT{d}_{h}",
                            name=f"oT{d}_{h}")
               for h in range(HPC)] for d in range(2)]

        # ---------- filler machinery ----------
        fillers = []
        pending_fin = [None]

        def run_pending():
            if pending_fin[0] is not None:
                f = pending_fin[0]
                pending_fin[0] = None
                f()

        def pop_fillers(k, budget=None):
            n = min(k, len(fillers))
            if budget is not None:
                n = min(n, budget[0])
                budget[0] -= n
            if n:
                fillers.pop(0)()
            run_pending()
            for _ in range(n - 1):
                fillers.pop(0)()

        def drain_fillers():
            if fillers:
                fillers.pop(0)()
            run_pending()
            while fillers:
                fillers.pop(0)()

        # ---------- projection helpers ----------
        def qproj_quarter(j, h, qps, kq):
            def emit():
                for kb in range(4 * kq, 4 * kq + 4):
                    nc.tensor.matmul(out=qps,
                                     lhsT=wq_sb[:, h, kb, :],
                                     rhs=x_all[:, j, kb, :],
                                     start=(kb == 0), stop=(kb == 15))
                if kq == 3:
                    nc.scalar.activation(out=qT4[h][:, j, :], in_=qps,
                                         func=mybir.ActivationFunctionType.Identity,
                                         bias=bq_sb[:, h:h + 1], scale=1.0)
            return emit

        def emit_qproj_pair(j, h0, h1):
            qpp = psum.tile([128, 2, 512], F32, tag="sps", bufs=2,
                            name=f"qpp{j}_{h0}")
            for pl, h in ((0, h0), (1, h1)):
                for kq in range(4):
                    qproj_quarter(j, h, qpp[:, pl, :], kq)()

        def vtrans(j, vplane):
            vt_sb = work.tile([128, 512], BF16, tag="vt", bufs=2,
                              name=f"vt{j}")
            nc.scalar.activation(out=vt_sb, in_=vplane,
                                 func=mybir.ActivationFunctionType.Identity,
                                 bias=bv_sb[:, 0:1], scale=1.0)
            vtp = psum.tile([128, 512], BF16, tag="op", bufs=2, name=f"vtp{j}")
            for sub in range(4):
                nc.tensor.transpose(vtp[:, sub * 128:(sub + 1) * 128],
                                    vt_sb[:, sub * 128:(sub + 1) * 128],
                                    ident)
            for sub in range(4):
                nc.vector.tensor_copy(out=v_sb[4 * j + sub],
                                      in_=vtp[:, sub * 128:(sub + 1) * 128])

        def emit_proj_slice0():
            """K/V projections for slice 0, matmuls interleaved at kb
            granularity in DMA arrival order, padded with dummy matmuls to
            keep the PE clock warm while x streams in."""
            kvp = psum.tile([128, 2, 512], F32, tag="sps", bufs=2, name="kvp0")
            q01 = psum.tile([128, 2, 512], F32, tag="sps", bufs=2, name="q01e")
            for kb in range(16):
                st, sp = (kb == 0), (kb == 15)
                nc.tensor.matmul(out=kvp[:, 0, :], lhsT=wk_sb[:, kb, :],
                                 rhs=x_all[:, 0, kb, :], start=st, stop=sp)
                nc.tensor.matmul(out=kvp[:, 1, :], lhsT=wv_sb[:, kb, :],
                                 rhs=x_all[:, 0, kb, :], start=st, stop=sp)
                if kb > 1:
                    nc.tensor.matmul(out=wps[:, kb % 2, :], lhsT=ones32,
                                     rhs=scr512, start=True, stop=True)
                # interleave Q(0,h0) quarters over already-arrived kb blocks
                if kb == 11:
                    qproj_quarter(0, 0, q01[:, 0, :], 0)()
                elif kb == 13:
                    qproj_quarter(0, 0, q01[:, 0, :], 1)()
                elif kb == 15:
                    qproj_quarter(0, 0, q01[:, 0, :], 2)()
            qproj_quarter(0, 0, q01[:, 0, :], 3)()
            nc.scalar.activation(out=kT[:, 0:512], in_=kvp[:, 0, :],
                                 func=mybir.ActivationFunctionType.Identity,
                                 bias=bk_sb[:, 0:1], scale=1.0)
            vtrans(0, kvp[:, 1, :])
            for kq in range(4):
                qproj_quarter(0, 1, q01[:, 1, :], kq)()

        def emit_kvproj(j):
            sl = slice(j * 512, (j + 1) * 512)
            kvp = psum.tile([128, 2, 512], F32, tag="sps", bufs=2,
                            name=f"kvp{j}")
            for kb in range(16):
                nc.tensor.matmul(out=kvp[:, 0, :], lhsT=wk_sb[:, kb, :],
                                 rhs=x_all[:, j, kb, :],
                                 start=(kb == 0), stop=(kb == 15))
            for kb in range(16):
                nc.tensor.matmul(out=kvp[:, 1, :], lhsT=wv_sb[:, kb, :],
                                 rhs=x_all[:, j, kb, :],
                                 start=(kb == 0), stop=(kb == 15))
            nc.scalar.activation(out=kT[:, sl], in_=kvp[:, 0, :],
                                 func=mybir.ActivationFunctionType.Identity,
                                 bias=bk_sb[:, 0:1], scale=1.0)
            vtrans(j, kvp[:, 1, :])

        # ---------- output projection units (filler fodder) ----------
        _ostg = {}

        def outproj_unit(jj, tt, n, heads, key):
            # matmuls (contract over `heads`) + staging copy + DMA per tt
            def emit():
                if key not in _ostg:
                    _ostg[key] = work.tile([128, 4, 512], BF16, tag="ostg",
                                           bufs=2, name=f"ostg{key}")
                ostg = _ostg[key]
                ops = psum.tile([128, 512], F32, tag="op", bufs=2,
                                name=f"ops{key}_{n}")
                sub = tt % 4
                for i, h in enumerate(heads):
                    nc.tensor.matmul(
                        out=ops,
                        lhsT=oT[jj % 2][h][:, sub * 128:(sub + 1) * 128],
                        rhs=wo_sb[:, h, n * 512:(n + 1) * 512],
                        start=(i == 0), stop=(i == len(heads) - 1))
                if n % 2 == 0:
                    nc.vector.tensor_copy(out=ostg[:, n, :], in_=ops)
                else:
                    nc.scalar.copy(out=ostg[:, n, :], in_=ops)
                if n == 3:
                    nc.gpsimd.dma_start(
                        out=part[tt * 128:(tt + 1) * 128, :], in_=ostg)
            return emit

        def queue_outproj(j):
            for sub in range(4):
                tt = 4 * j + sub
                for n in range(4):
                    fillers.append(
                        outproj_unit(j, tt, n, heads=(0, 1, 2, 3), key=tt))

        def tail_outproj():
            """Slice-3 outproj on freed attention PSUM pair tiles; one
            [128,2048] DMA per row-block."""
            for sub in range(4):
                tt = 12 + sub
                ostg = work.tile([128, 4, 512], BF16, tag="ostg", bufs=2,
                                 name=f"ostgT{tt}")
                for half in range(2):
                    opp = psum.tile([128, 2, 512], F32, tag="sps", bufs=2,
                                    name=f"oppT{tt}_{half}")
                    for pl in range(2):
                        n = 2 * half + pl
                        for i, h in enumerate((0, 1, 2, 3)):
                            nc.tensor.matmul(
                                out=opp[:, pl, :],
                                lhsT=oT[1][h][:, (tt % 4) * 128:
                                              (tt % 4 + 1) * 128],
                                rhs=wo_sb[:, h, n * 512:(n + 1) * 512],
                                start=(i == 0), stop=(i == 3))
                    dst = ostg[:, 2 * half:2 * half + 2, :]
                    if (sub + half) % 2 == 0:
                        nc.vector.tensor_copy(out=dst, in_=opp)
                    else:
                        nc.scalar.copy(out=dst, in_=opp)
                nc.gpsimd.dma_start(out=part[tt * 128:(tt + 1) * 128, :],
                                    in_=ostg)

        # ---------- attention ----------
        def emit_attention_head(j, h, fill_rate, budget=None):
            """Attention for head h over tq-slice j, tk blocks 0..4j+3.
            S^T per tk-block PAIR into a [128,2,512] PSUM pair tile with a
            single fused exp; the 4 diagonal blocks are column-trimmed and
            their S matmuls issued before the exp/mask/PV chains."""
            otps = psum.tile([128, 512], F32, tag="acc", bufs=2,
                             name=f"otps{h}_{j}")
            racc = work.tile([128, 512], F32, tag="racc", bufs=2,
                             name=f"racc{h}_{j}")

            def pv_mm(tkb, pt_ap, o_off, start, stop):
                nc.tensor.matmul(out=otps[:, o_off:512], lhsT=v_sb[tkb],
                                 rhs=pt_ap, start=start, stop=stop,
                                 skip_group_check=True)

            pps = []
            racc_init = [False]

            def racc_accum(ap):
                if not racc_init[0]:
                    nc.vector.tensor_copy(out=racc, in_=ap)
                    racc_init[0] = True
                else:
                    nc.vector.tensor_add(out=racc, in0=racc, in1=ap)

            # --- non-diagonal block pairs ---
            for p in range(2 * j):
                spair = psum.tile([128, 2, 512], F32, tag="sps", bufs=2,
                                  name=f"sp{h}_{j}_{p}")
                for q in range(2):
                    nc.tensor.matmul(
                        out=spair[:, q, :],
                        lhsT=kT[:, (2 * p + q) * 128:(2 * p + q + 1) * 128],
                        rhs=qT4[h][:, j, :], start=True, stop=True)
                ptp = work.tile([128, 2, 512], BF16, tag="pt", bufs=4,
                                name=f"pt{h}_{j}_{p}")
                nc.scalar.activation(out=ptp, in_=spair,
                                     func=mybir.ActivationFunctionType.Exp,
                                     scale=SCALE)
                pv_mm(2 * p, ptp[:, 0, :], 0, start=(p == 0), stop=False)
                pv_mm(2 * p + 1, ptp[:, 1, :], 0, start=False, stop=False)
                pp = work.tile([128, 512], BF16, tag="ppair", bufs=4,
                               name=f"pp{h}_{j}_{p}")
                nc.vector.tensor_add(out=pp, in0=ptp[:, 0, :],
                                     in1=ptp[:, 1, :])
                pps.append(pp)
                if len(pps) == 2:
                    qs = work.tile([128, 512], BF16, tag="qsum", bufs=2,
                                   name=f"qs{h}_{j}_{p}")
                    nc.vector.tensor_add(out=qs, in0=pps[0], in1=pps[1])
                    pps.clear()
                    racc_accum(qs)
                pop_fillers(fill_rate, budget)
            if pps:
                racc_accum(pps[0])
                pps.clear()

            # --- diagonal blocks r=0..3, column-trimmed ---
            # All four S matmuls first (two pair tiles), then fillers, then
            # the exp/mask/PV chains so the PE is never inside the chain.
            base = 4 * j
            dp = [psum.tile([128, 2, 512], F32, tag="sps", bufs=2,
                            name=f"dp{h}_{j}_{i}") for i in range(2)]
            for r in range(4):
                w = 512 - 128 * r
                nc.tensor.matmul(
                    out=dp[r // 2][:, r % 2, 0:w],
                    lhsT=kT[:, (base + r) * 128:(base + r + 1) * 128],
                    rhs=qT4[h][:, j, 128 * r:512],
                    start=True, stop=True)
            pop_fillers(fill_rate, budget)
            ptd = [work.tile([128, 2, 512], BF16, tag="pt", bufs=4,
                             name=f"ptd{h}_{j}_{i}") for i in range(2)]
            for r in range(4):
                w = 512 - 128 * r
                nc.scalar.activation(
                    out=ptd[r // 2][:, r % 2, 0:w],
                    in_=dp[r // 2][:, r % 2, 0:w],
                    func=mybir.ActivationFunctionType.Exp, scale=SCALE)
            for r in range(4):
                nc.gpsimd.affine_select(
                    out=ptd[r // 2][:, r % 2, 0:128],
                    in_=ptd[r // 2][:, r % 2, 0:128],
                    compare_op=mybir.AluOpType.is_ge,
                    fill=0.0, base=0,
                    pattern=[[1, 128]], channel_multiplier=-1)
            for r in range(4):
                w = 512 - 128 * r
                w_off = 128 * r
                pv_mm(base + r, ptd[r // 2][:, r % 2, 0:w], w_off,
                      start=(j == 0 and r == 0), stop=(r == 3))
                if r == 0:
                    racc_accum(ptd[0][:, 0, :])
                else:
                    nc.vector.tensor_add(out=racc[:, w_off:512],
                                         in0=racc[:, w_off:512],
                                         in1=ptd[r // 2][:, r % 2, 0:w])
                if r % 2 == 1:
                    pop_fillers(fill_rate, budget)

            # --- denominator: partition reduction ---
            if ROWSUM_GPSIMD:
                def finish():
                    rs = work.tile([128, 512], F32, tag="rs", bufs=2,
                                   name=f"rs{h}_{j}")
                    nc.gpsimd.partition_all_reduce(
                        rs, racc, 128, bass_isa.ReduceOp.add)
                    rinv = work.tile([128, 512], F32, tag="rinv", bufs=2,
                                     name=f"rinv{h}_{j}")
                    nc.vector.reciprocal_approx_fast(rinv, rs)
                    nc.vector.tensor_mul(out=oT[j % 2][h], in0=otps,
                                         in1=rinv)
                return finish
            racc16 = work.tile([128, 512], BF16, tag="racc16", bufs=2,
                               name=f"racc16{h}_{j}")
            nc.vector.tensor_copy(out=racc16, in_=racc)

            def finish():
                rsb = psum.tile([128, 512], F32, tag="op", bufs=2,
                                name=f"rsb{h}_{j}")
                nc.tensor.matmul(out=rsb, lhsT=ones32, rhs=racc16,
                                 start=True, stop=True)
                rinv = work.tile([128, 512], F32, tag="rinv", bufs=2,
                                 name=f"rinv{h}_{j}")
                nc.vector.reciprocal_approx_fast(rinv, rsb)
                nc.vector.tensor_mul(out=oT[j % 2][h], in0=otps, in1=rinv)
            return finish

        # ---------- main schedule ----------
        # Phase A: stream-paced projections (KV+Q per slice, in x arrival
        # order) with attention(0) filling the xs1 arrival gap.
        emit_proj_slice0()
        emit_qproj_pair(0, 2, 3)
        pending_fin[0] = emit_attention_head(0, 0, fill_rate=1)
        pending_fin[0] = emit_attention_head(0, 1, fill_rate=1)
        pending_fin[0] = emit_attention_head(0, 2, fill_rate=1)
        fin0 = emit_attention_head(0, 3, fill_rate=1)
        for js in range(1, 4):
            emit_kvproj(js)
            if js == 1:
                fin0()
            emit_qproj_pair(js, 0, 1)
            emit_qproj_pair(js, 2, 3)
        # Phase B: attention(1..3) with outproj fillers.
        fin3 = [None]
        for j in range(1, 4):
            # flush the deferred normalize of head (j-1, 3) BEFORE any
            # outproj(j-1) unit can pop -- those units read oT[(j-1)%2]
            run_pending()
            queue_outproj(j - 1)
            if j < 3:
                pending_fin[0] = emit_attention_head(j, 0, fill_rate=1)
                pending_fin[0] = emit_attention_head(j, 1, fill_rate=1)
                pending_fin[0] = emit_attention_head(j, 2, fill_rate=1)
                pending_fin[0] = emit_attention_head(j, 3, fill_rate=1)
            else:
                pending_fin[0] = emit_attention_head(3, 0, fill_rate=1,
                                                     budget=[5])
                pending_fin[0] = emit_attention_head(3, 1, fill_rate=1,
                                                     budget=[4])
                pending_fin[0] = emit_attention_head(3, 2, fill_rate=1,
                                                     budget=[3])
                fin3[0] = emit_attention_head(3, 3, fill_rate=1, budget=[2])
                run_pending()
                fin3[0]()
                fin3[0] = None
                drain_fillers()
                tail_outproj()
        drain_fillers()

    nc.compile()
    return nc


def _get_nc():
    if "nc" not in _CACHE:
        _CACHE["nc"] = _build_nc()
    return _CACHE["nc"]


def _bf16(a):
    return np.ascontiguousarray(a.astype(ml_dtypes.bfloat16))


def _tile16(a):
    # [2048, C] -> [128, 16, C]   (rows kb*128+p -> [p, kb, :])
    c = a.shape[1]
    return np.ascontiguousarray(
        a.reshape(16, 128, c).transpose(1, 0, 2))


def kernel(x, Wq, bq, Wk, bk, Wv, bv, Wo, bo, **kw):
    x = np.asarray(x, dtype=np.float32)
    Wq = np.asarray(Wq, dtype=np.float32)
    Wk = np.asarray(Wk, dtype=np.float32)
    Wv = np.asarray(Wv, dtype=np.float32)
    Wo = np.asarray(Wo, dtype=np.float32)
    bq = np.asarray(bq, dtype=np.float32)
    bk = np.asarray(bk, dtype=np.float32)
    bv = np.asarray(bv, dtype=np.float32)
    bo = np.asarray(bo, dtype=np.float32)

    nc = _get_nc()
    xt_b = [np.ascontiguousarray(
        _tile16(_bf16(x[b].T)).reshape(128, 16, 4, 512).transpose(0, 2, 1, 3))
        for b in range(B)]
    in_maps = []
    for c in range(NCORES):
        b = c // 4
        q = c % 4
        hs = q * HPC * DH          # column start in Wq / row start in Wo
        kv = q // 2
        bq_m = np.ascontiguousarray(
            bq[hs:hs + HPC * DH].reshape(HPC, DH).T)          # [128, 4]
        bk_m = np.ascontiguousarray(
            bk[kv * DH:(kv + 1) * DH].reshape(DH, 1))         # [128, 1]
        bv_m = np.ascontiguousarray(
            bv[kv * DH:(kv + 1) * DH].reshape(DH, 1))         # [128, 1]
        in_maps.append({
            "xt": xt_b[b],
            "wq": np.ascontiguousarray(
                _tile16(_bf16(Wq[:, hs:hs + HPC * DH]))
                .reshape(128, 16, HPC, DH).transpose(0, 2, 1, 3)),
            "wk": _tile16(_bf16(Wk[:, kv * DH:(kv + 1) * DH])),
            "wv": _tile16(_bf16(Wv[:, kv * DH:(kv + 1) * DH])),
            "wo": np.ascontiguousarray(
                _bf16(Wo[hs:hs + HPC * DH, :]).reshape(HPC, 128, D)
                .transpose(1, 0, 2)),
            "bqm": bq_m,
            "bkm": bk_m,
            "bvm": bv_m,
        })

    res = run_bass_kernel_spmd(nc, in_maps, list(range(NCORES)),
                               **kw.get("_run_kwargs", {}))
    if kw.get("_return_res"):
        return res
    parts = [np.asarray(res.results[c]["part"]).astype(np.float32)
             for c in range(NCORES)]
    out = np.empty((B, T, D), dtype=np.float32)
    for b in range(B):
        acc = parts[4 * b]
        for q in range(1, 4):
            acc += parts[4 * b + q]
        out[b] = acc + bo[None, :]
    return out


# revision 20
# speedup vs baseline: 1.1398x; 1.1398x over previous
"""GQA kernel for Trainium2, 8 NeuronCores.

Problem: B=2, T=2048, D=2048, 16 query heads / 2 KV heads, d_head=128, causal.

Sharding: core c -> batch b = c//4, head-quarter q = c%4 (query heads
4q..4q+3, kv head q//2). Each core computes its 4 heads' attention and a
partial output projection (its Wo rows); host sums the 4 partials per batch
and adds bo.

Host marshalling: weights and x are pre-cast to bf16 and pre-tiled so each
logical input lands with one multi-dim DMA per priority chunk:
  xt [128, 16, 2048] = x[b].T tiled as (p, kb, t)
  wq [128, 16, 512], wk/wv [128, 16, 128]  (p, kb, cols)
  wo [128, 4, 2048]                        (p=dh, h, n)
Inputs stream on FOUR queues (sync/gpsimd/vector/scalar) in approximate
consumption order (KB blocks consumed 0..15): the early-engine queues
(vector/scalar) only carry startup chunks, before their first compute.

PE warm-up: the HAM clock gate keeps the PE at 1.2 GHz until ~3.4us of
sustained activity; dummy matmuls bridge the initial DMA wait so real
matmuls start at 2.4 GHz.

On-core dataflow (bf16 matmuls, fp32 PSUM):
per 512-wide t-slice j: K/V projections into one [128,2,512] PSUM pair
tile (ACT epilogues w/ bias); V PE-transposed to natural layout; Q heads
pairwise in PSUM pair tiles.  Attention per head h over tk blocks: S^T
computed per tk-block PAIR into a [128,2,512] PSUM pair tile; ONE fused
exp per pair (ACT reads both banks in one ACTIVATE); causal diagonal
blocks column-trimmed with all four S matmuls issued before their
exp/mask/PV chains (PE never sits in the chain); 128x128 affine_select
masks on gpsimd only for the true diagonal; PV accumulated into otps;
softmax denominator: bf16 pair-tree + fp32 racc on DVE, bf16 cast, one
ones-matmul partition reduction per (h,j); normalize (reciprocal+mul) on
DVE.  Per-head normalize is deferred into the next head's stream.
Output projection for slice j is cut into 4-matmul units used as PE
filler inside slice j+1's attention rounds; a few units are RESERVED to
cover the final head's normalize chain; slice-3 outproj uses freed
PSUM pair tiles and one [128,2048] DMA per row-block.
"""

import numpy as np
import ml_dtypes
from contextlib import ExitStack

import concourse.bass as bass
from concourse import bacc
import concourse.mybir as mybir
import concourse.tile as tile
from concourse.bass_utils import run_bass_kernel_spmd
from concourse.masks import make_identity
from concourse import bass_isa

F32 = mybir.dt.float32
F32R = mybir.dt.float32r
BF16 = mybir.dt.bfloat16

D = 2048
T = 2048
DH = 128
B = 2
HPC = 4            # query heads per core
NCORES = 8
SCALE = 1.0 / float(np.sqrt(128.0))
N_WARM = 9         # PE warm-up dummy matmuls (512-col)
ROWSUM_GPSIMD = False  # denominator partition-reduce on gpsimd vs PE matmul

_CACHE = {}


def _build_nc():
    nc = bacc.Bacc("TRN2", target_bir_lowering=False, debug=False,
                   num_devices=NCORES)

    xt = nc.dram_tensor("xt", [128, 4, 16, 512], BF16, kind="ExternalInput")
    wq = nc.dram_tensor("wq", [128, HPC, 16, DH], BF16, kind="ExternalInput")
    wk = nc.dram_tensor("wk", [128, 16, DH], BF16, kind="ExternalInput")
    wv = nc.dram_tensor("wv", [128, 16, DH], BF16, kind="ExternalInput")
    wo = nc.dram_tensor("wo", [128, HPC, D], BF16, kind="ExternalInput")
    bqm = nc.dram_tensor("bqm", [DH, HPC], F32, kind="ExternalInput")
    bkm = nc.dram_tensor("bkm", [DH, 1], F32, kind="ExternalInput")
    bvm = nc.dram_tensor("bvm", [DH, 1], F32, kind="ExternalInput")
    part = nc.dram_tensor("part", [T, D], BF16, kind="ExternalOutput")

    with ExitStack() as ctx:
        tc = ctx.enter_context(tile.TileContext(nc))
        persist = ctx.enter_context(tc.tile_pool(name="persist", bufs=1))
        work = ctx.enter_context(tc.tile_pool(name="work", bufs=2))
        psum = ctx.enter_context(tc.tile_pool(name="psum", bufs=2, space="PSUM"))

        # ---- constants ----
        ones32 = persist.tile([128, 128], BF16, tag="ones32", name="ones32")
        nc.vector.memset(ones32, 1.0)
        scr512 = persist.tile([128, 512], BF16, tag="scr512", name="scr512")
        nc.vector.memset(scr512, 0.0)
        ident = persist.tile([128, 128], BF16, tag="ident", name="ident")
        make_identity(nc, ident)
        bq_sb = persist.tile([DH, HPC], F32, tag="bq", name="bq_sb")
        bk_sb = persist.tile([DH, 1], F32, tag="bk", name="bk_sb")
        bv_sb = persist.tile([DH, 1], F32, tag="bv", name="bv_sb")

        # ---- persistent input tiles ----
        x_all = persist.tile([128, 4, 16, 512], BF16, tag="x_all", name="x_all")
        wq_sb = persist.tile([128, HPC, 16, DH], BF16, tag="wq", name="wq_sb")
        wk_sb = persist.tile([128, 16, DH], BF16, tag="wk", name="wk_sb")
        wv_sb = persist.tile([128, 16, DH], BF16, tag="wv", name="wv_sb")
        wo_sb = persist.tile([128, HPC, D], BF16, tag="wo", name="wo_sb")

        # ---- input DMAs ----
        # Single fast queue (gpsimd) in exact consumption order; all DMAs
        # are contiguous on both sides (x is slice-major, wq head-major).
        # Only wo (needed ~70us later) rides the slow sync queue.
        nc.gpsimd.dma_start(out=bq_sb, in_=bqm[:, :])
        nc.gpsimd.dma_start(out=bk_sb, in_=bkm[:, :])
        nc.gpsimd.dma_start(out=bv_sb, in_=bvm[:, :])
        nc.gpsimd.dma_start(out=wk_sb, in_=wk[:, :, :])
        nc.gpsimd.dma_start(out=wv_sb, in_=wv[:, :, :])
        for duo in range(6):
            a = slice(2 * duo, 2 * duo + 2)
            nc.gpsimd.dma_start(out=x_all[:, 0, a, :], in_=xt[:, 0, a, :])
        nc.gpsimd.dma_start(out=wq_sb[:, 0], in_=wq[:, 0])
        nc.gpsimd.dma_start(out=x_all[:, 0, 12:14, :], in_=xt[:, 0, 12:14, :])
        nc.gpsimd.dma_start(out=x_all[:, 0, 14:16, :], in_=xt[:, 0, 14:16, :])
        for h in range(1, 4):
            nc.gpsimd.dma_start(out=wq_sb[:, h], in_=wq[:, h])
        for js in range(1, 4):
            nc.gpsimd.dma_start(out=x_all[:, js, 0:8, :], in_=xt[:, js, 0:8, :])
            nc.gpsimd.dma_start(out=x_all[:, js, 8:16, :],
                                in_=xt[:, js, 8:16, :])
        nc.sync.dma_start(out=wo_sb[:, 0:2, :], in_=wo[:, 0:2, :])
        nc.sync.dma_start(out=wo_sb[:, 2:4, :], in_=wo[:, 2:4, :])

        # warm the ACT exp table-set (~2.7us) during the initial DMA wait
        warm = persist.tile([128, 1], F32, tag="warm", name="warm")
        nc.scalar.activation(out=warm, in_=ident[:, 0:1],
                             func=mybir.ActivationFunctionType.Exp)

        # ---- PE warm-up dummies: keep the HAM clock gate open ----
        wps = psum.tile([128, 2, 512], F32, tag="sps", bufs=2, name="wps")
        for i in range(N_WARM):
            nc.tensor.matmul(out=wps[:, i % 2, :], lhsT=ones32, rhs=scr512,
                             start=True, stop=True)

        # ---- persistent activations ----
        kT = persist.tile([128, T], BF16, tag="kT", name="kT")
        v_sb = [persist.tile([128, DH], BF16, tag=f"v{t}", name=f"v{t}")
                for t in range(16)]
        # per-slice q and o (o double-buffered: outproj(j) runs during j+1)
        qT4 = [persist.tile([128, 4, 512], BF16, tag=f"qT{h}", name=f"qT{h}")
               for h in range(HPC)]
        oT = [[persist.tile([128, 512], BF16, tag=f"oT{d}_{h}",
                            name=f"oT{d}_{h}")
               for h in range(HPC)] for d in range(2)]

        # ---------- filler machinery ----------
        fillers = []
        pending_fin = [None]

        def run_pending():
            if pending_fin[0] is not None:
                f = pending_fin[0]
                pending_fin[0] = None
                f()

        def pop_fillers(k, budget=None):
            n = min(k, len(fillers))
            if budget is not None:
                n = min(n, budget[0])
                budget[0] -= n
            if n:
                fillers.pop(0)()
            run_pending()
            for _ in range(n - 1):
                fillers.pop(0)()

        def drain_fillers():
            if fillers:
                fillers.pop(0)()
            run_pending()
            while fillers:
                fillers.pop(0)()

        # ---------- projection helpers ----------
        def qproj_quarter(j, h, qps, kq):
            def emit():
                for kb in range(4 * kq, 4 * kq + 4):
                    nc.tensor.matmul(out=qps,
                                     lhsT=wq_sb[:, h, kb, :],
                                     rhs=x_all[:, j, kb, :],
                                     start=(kb == 0), stop=(kb == 15))
                if kq == 3:
                    nc.scalar.activation(out=qT4[h][:, j, :], in_=qps,
                                         func=mybir.ActivationFunctionType.Identity,
                                         bias=bq_sb[:, h:h + 1], scale=1.0)
            return emit

        def emit_qproj_pair(j, h0, h1):
            qpp = psum.tile([128, 2, 512], F32, tag="sps", bufs=2,
                            name=f"qpp{j}_{h0}")
            for pl, h in ((0, h0), (1, h1)):
                for kq in range(4):
                    qproj_quarter(j, h, qpp[:, pl, :], kq)()

        def vtrans(j, vplane):
            vt_sb = work.tile([128, 512], BF16, tag="vt", bufs=2,
                              name=f"vt{j}")
            nc.scalar.activation(out=vt_sb, in_=vplane,
                                 func=mybir.ActivationFunctionType.Identity,
                                 bias=bv_sb[:, 0:1], scale=1.0)
            vtp = psum.tile([128, 512], BF16, tag="op", bufs=2, name=f"vtp{j}")
            for sub in range(4):
                nc.tensor.transpose(vtp[:, sub * 128:(sub + 1) * 128],
                                    vt_sb[:, sub * 128:(sub + 1) * 128],
                                    ident)
            for sub in range(4):
                nc.vector.tensor_copy(out=v_sb[4 * j + sub],
                                      in_=vtp[:, sub * 128:(sub + 1) * 128])

        def emit_proj_slice0():
            """K/V projections for slice 0, matmuls interleaved at kb
            granularity in DMA arrival order, padded with dummy matmuls to
            keep the PE clock warm while x streams in."""
            kvp = psum.tile([128, 2, 512], F32, tag="sps", bufs=2, name="kvp0")
            q01 = psum.tile([128, 2, 512], F32, tag="sps", bufs=2, name="q01e")
            for kb in range(16):
                st, sp = (kb == 0), (kb == 15)
                nc.tensor.matmul(out=kvp[:, 0, :], lhsT=wk_sb[:, kb, :],
                                 rhs=x_all[:, 0, kb, :], start=st, stop=sp)
                nc.tensor.matmul(out=kvp[:, 1, :], lhsT=wv_sb[:, kb, :],
                                 rhs=x_all[:, 0, kb, :], start=st, stop=sp)
                if kb > 1:
                    nc.tensor.matmul(out=wps[:, kb % 2, :], lhsT=ones32,
                                     rhs=scr512, start=True, stop=True)
                # interleave Q(0,h0) quarters over already-arrived kb blocks
                if kb == 11:
                    qproj_quarter(0, 0, q01[:, 0, :], 0)()
                elif kb == 13:
                    qproj_quarter(0, 0, q01[:, 0, :], 1)()
                elif kb == 15:
                    qproj_quarter(0, 0, q01[:, 0, :], 2)()
            qproj_quarter(0, 0, q01[:, 0, :], 3)()
            nc.scalar.activation(out=kT[:, 0:512], in_=kvp[:, 0, :],
                                 func=mybir.ActivationFunctionType.Identity,
                                 bias=bk_sb[:, 0:1], scale=1.0)
            vtrans(0, kvp[:, 1, :])
            for kq in range(4):
                qproj_quarter(0, 1, q01[:, 1, :], kq)()

        def emit_kvproj(j):
            sl = slice(j * 512, (j + 1) * 512)
            kvp = psum.tile([128, 2, 512], F32, tag="sps", bufs=2,
                            name=f"kvp{j}")
            for kb in range(16):
                nc.tensor.matmul(out=kvp[:, 0, :], lhsT=wk_sb[:, kb, :],
                                 rhs=x_all[:, j, kb, :],
                                 start=(kb == 0), stop=(kb == 15))
            for kb in range(16):
                nc.tensor.matmul(out=kvp[:, 1, :], lhsT=wv_sb[:, kb, :],
                                 rhs=x_all[:, j, kb, :],
                                 start=(kb == 0), stop=(kb == 15))
            nc.scalar.activation(out=kT[:, sl], in_=kvp[:, 0, :],
                                 func=mybir.ActivationFunctionType.Identity,
                                 bias=bk_sb[:, 0:1], scale=1.0)
            vtrans(j, kvp[:, 1, :])

        # ---------- output projection units (filler fodder) ----------
        _ostg = {}

        def outproj_unit(jj, tt, n, heads, key):
            # matmuls (contract over `heads`) + staging copy + DMA per tt
            def emit():
                if key not in _ostg:
                    _ostg[key] = work.tile([128, 4, 512], BF16, tag="ostg",
                                           bufs=2, name=f"ostg{key}")
                ostg = _ostg[key]
                ops = psum.tile([128, 512], F32, tag="op", bufs=2,
                                name=f"ops{key}_{n}")
                sub = tt % 4
                for i, h in enumerate(heads):
                    nc.tensor.matmul(
                        out=ops,
                        lhsT=oT[jj % 2][h][:, sub * 128:(sub + 1) * 128],
                        rhs=wo_sb[:, h, n * 512:(n + 1) * 512],
                        start=(i == 0), stop=(i == len(heads) - 1))
                if n % 2 == 0:
                    nc.vector.tensor_copy(out=ostg[:, n, :], in_=ops)
                else:
                    nc.scalar.copy(out=ostg[:, n, :], in_=ops)
                if n == 3:
                    nc.gpsimd.dma_start(
                        out=part[tt * 128:(tt + 1) * 128, :], in_=ostg)
            return emit

        def queue_outproj(j):
            for sub in range(4):
                tt = 4 * j + sub
                for n in range(4):
                    fillers.append(
                        outproj_unit(j, tt, n, heads=(0, 1, 2, 3), key=tt))

        def tail_outproj():
            """Slice-3 outproj on freed attention PSUM pair tiles; one
            [128,2048] DMA per row-block."""
            for sub in range(4):
                tt = 12 + sub
                ostg = work.tile([128, 4, 512], BF16, tag="ostg", bufs=2,
                                 name=f"ostgT{tt}")
                for half in range(2):
                    opp = psum.tile([128, 2, 512], F32, tag="sps", bufs=2,
                                    name=f"oppT{tt}_{half}")
                    for pl in range(2):
                        n = 2 * half + pl
                        for i, h in enumerate((0, 1, 2, 3)):
                            nc.tensor.matmul(
                                out=opp[:, pl, :],
                                lhsT=oT[1][h][:, (tt % 4) * 128:
                                              (tt % 4 + 1) * 128],
                                rhs=wo_sb[:, h, n * 512:(n + 1) * 512],
                                start=(i == 0), stop=(i == 3))
                    dst = ostg[:, 2 * half:2 * half + 2, :]
                    if (sub + half) % 2 == 0:
                        nc.vector.tensor_copy(out=dst, in_=opp)
                    else:
                        nc.scalar.copy(out=dst, in_=opp)
                nc.gpsimd.dma_start(out=part[tt * 128:(tt + 1) * 128, :],
                                    in_=ostg)

        # ---------- attention ----------
        def emit_attention_head(j, h, fill_rate, budget=None):
            """Attention for head h over tq-slice j, tk blocks 0..4j+3.
            S^T per tk-block PAIR into a [128,2,512] PSUM pair tile with a
            single fused exp; the 4 diagonal blocks are column-trimmed and
            their S matmuls issued before the exp/mask/PV chains."""
            otps = psum.tile([128, 512], F32, tag="acc", bufs=2,
                             name=f"otps{h}_{j}")
            racc = work.tile([128, 512], F32, tag="racc", bufs=2,
                             name=f"racc{h}_{j}")

            def pv_mm(tkb, pt_ap, o_off, start, stop):
                nc.tensor.matmul(out=otps[:, o_off:512], lhsT=v_sb[tkb],
                                 rhs=pt_ap, start=start, stop=stop,
                                 skip_group_check=True)

            pps = []
            racc_init = [False]

            def racc_accum(ap):
                if not racc_init[0]:
                    nc.vector.tensor_copy(out=racc, in_=ap)
                    racc_init[0] = True
                else:
                    nc.vector.tensor_add(out=racc, in0=racc, in1=ap)

            # --- non-diagonal block pairs ---
            for p in range(2 * j):
                spair = psum.tile([128, 2, 512], F32, tag="sps", bufs=2,
                                  name=f"sp{h}_{j}_{p}")
                for q in range(2):
                    nc.tensor.matmul(
                        out=spair[:, q, :],
                        lhsT=kT[:, (2 * p + q) * 128:(2 * p + q + 1) * 128],
                        rhs=qT4[h][:, j, :], start=True, stop=True)
                ptp = work.tile([128, 2, 512], BF16, tag="pt", bufs=4,
                                name=f"pt{h}_{j}_{p}")
                nc.scalar.activation(out=ptp, in_=spair,
                                     func=mybir.ActivationFunctionType.Exp,
                                     scale=SCALE)
                pv_mm(2 * p, ptp[:, 0, :], 0, start=(p == 0), stop=False)
                pv_mm(2 * p + 1, ptp[:, 1, :], 0, start=False, stop=False)
                pp = work.tile([128, 512], BF16, tag="ppair", bufs=4,
                               name=f"pp{h}_{j}_{p}")
                nc.vector.tensor_add(out=pp, in0=ptp[:, 0, :],
                                     in1=ptp[:, 1, :])
                pps.append(pp)
                if len(pps) == 2:
                    qs = work.tile([128, 512], BF16, tag="qsum", bufs=2,
                                   name=f"qs{h}_{j}_{p}")
                    nc.vector.tensor_add(out=qs, in0=pps[0], in1=pps[1])
                    pps.clear()
                    racc_accum(qs)
                pop_fillers(fill_rate, budget)
            if pps:
                racc_accum(pps[0])
                pps.clear()

            # --- diagonal blocks r=0..3, column-trimmed ---
            # All four S matmuls first (two pair tiles), then fillers, then
            # the exp/mask/PV chains so the PE is never inside the chain.
            base = 4 * j
            dp = [psum.tile([128, 2, 512], F32, tag="sps", bufs=2,
                            name=f"dp{h}_{j}_{i}") for i in range(2)]
            for r in range(4):
                w = 512 - 128 * r
                nc.tensor.matmul(
                    out=dp[r // 2][:, r % 2, 0:w],
                    lhsT=kT[:, (base + r) * 128:(base + r + 1) * 128],
                    rhs=qT4[h][:, j, 128 * r:512],
                    start=True, stop=True)
            pop_fillers(fill_rate, budget)
            ptd = [work.tile([128, 2, 512], BF16, tag="pt", bufs=4,
                             name=f"ptd{h}_{j}_{i}") for i in range(2)]
            for r in range(4):
                w = 512 - 128 * r
                nc.scalar.activation(
                    out=ptd[r // 2][:, r % 2, 0:w],
                    in_=dp[r // 2][:, r % 2, 0:w],
                    func=mybir.ActivationFunctionType.Exp, scale=SCALE)
            for r in range(4):
                nc.gpsimd.affine_select(
                    out=ptd[r // 2][:, r % 2, 0:128],
                    in_=ptd[r // 2][:, r % 2, 0:128],
                    compare_op=mybir.AluOpType.is_ge,
                    fill=0.0, base=0,
                    pattern=[[1, 128]], channel_multiplier=-1)
            for r in range(4):
                w = 512 - 128 * r
                w_off = 128 * r
                pv_mm(base + r, ptd[r // 2][:, r % 2, 0:w], w_off,
                      start=(j == 0 and r == 0), stop=(r == 3))
                if r == 0:
                    racc_accum(ptd[0][:, 0, :])
                else:
                    nc.vector.tensor_add(out=racc[:, w_off:512],
                                         in0=racc[:, w_off:512],
                                         in1=ptd[r // 2][:, r % 2, 0:w])
                if r % 2 == 1:
                    pop_fillers(fill_rate, budget)

            # --- denominator: partition reduction ---
            if ROWSUM_GPSIMD:
                def finish():
                    rs = work.tile([128, 512], F32, tag="rs", bufs=2,
                                   name=f"rs{h}_{j}")
                    nc.gpsimd.partition_all_reduce(
                        rs, racc, 128, bass_isa.ReduceOp.add)
                    rinv = work.tile([128, 512], F32, tag="rinv", bufs=2,
                                     name=f"rinv{h}_{j}")
                    nc.vector.reciprocal_approx_fast(rinv, rs)
                    nc.vector.tensor_mul(out=oT[j % 2][h], in0=otps,
                                         in1=rinv)
                return finish
            racc16 = work.tile([128, 512], BF16, tag="racc16", bufs=2,
                               name=f"racc16{h}_{j}")
            nc.vector.tensor_copy(out=racc16, in_=racc)

            def finish():
                rsb = psum.tile([128, 512], F32, tag="op", bufs=2,
                                name=f"rsb{h}_{j}")
                nc.tensor.matmul(out=rsb, lhsT=ones32, rhs=racc16,
                                 start=True, stop=True)
                rinv = work.tile([128, 512], F32, tag="rinv", bufs=2,
                                 name=f"rinv{h}_{j}")
                nc.vector.reciprocal_approx_fast(rinv, rsb)
                nc.vector.tensor_mul(out=oT[j % 2][h], in0=otps, in1=rinv)
            return finish

        # ---------- main schedule ----------
        # Phase A: stream-paced projections (KV+Q per slice, in x arrival
        # order) with attention(0) filling the xs1 arrival gap.
        emit_proj_slice0()
        emit_qproj_pair(0, 2, 3)
        pending_fin[0] = emit_attention_head(0, 0, fill_rate=1)
        pending_fin[0] = emit_attention_head(0, 1, fill_rate=1)
        pending_fin[0] = emit_attention_head(0, 2, fill_rate=1)
        fin0 = emit_attention_head(0, 3, fill_rate=1)
        for js in range(1, 4):
            emit_kvproj(js)
            if js == 1:
                fin0()
            emit_qproj_pair(js, 0, 1)
            emit_qproj_pair(js, 2, 3)
        # Phase B: attention(1..3) with outproj fillers.
        fin3 = [None]
        for j in range(1, 4):
            # flush the deferred normalize of head (j-1, 3) BEFORE any
            # outproj(j-1) unit can pop -- those units read oT[(j-1)%2]
            run_pending()
            queue_outproj(j - 1)
            if j < 3:
                pending_fin[0] = emit_attention_head(j, 0, fill_rate=1)
                pending_fin[0] = emit_attention_head(j, 1, fill_rate=1)
                pending_fin[0] = emit_attention_head(j, 2, fill_rate=1)
                pending_fin[0] = emit_attention_head(j, 3, fill_rate=1)
            else:
                pending_fin[0] = emit_attention_head(3, 0, fill_rate=1,
                                                     budget=[5])
                pending_fin[0] = emit_attention_head(3, 1, fill_rate=1,
                                                     budget=[4])
                pending_fin[0] = emit_attention_head(3, 2, fill_rate=1,
                                                     budget=[3])
                fin3[0] = emit_attention_head(3, 3, fill_rate=1, budget=[2])
                run_pending()
                fin3[0]()
                fin3[0] = None
                drain_fillers()
                tail_outproj()
        drain_fillers()

    nc.compile()
    return nc


def _get_nc():
    if "nc" not in _CACHE:
        _CACHE["nc"] = _build_nc()
    return _CACHE["nc"]


def _bf16(a):
    return np.ascontiguousarray(a.astype(ml_dtypes.bfloat16))


def _tile16(a):
    # [2048, C] -> [128, 16, C]   (rows kb*128+p -> [p, kb, :])
    c = a.shape[1]
    return np.ascontiguousarray(
        a.reshape(16, 128, c).transpose(1, 0, 2))


def kernel(x, Wq, bq, Wk, bk, Wv, bv, Wo, bo, **kw):
    x = np.asarray(x, dtype=np.float32)
    Wq = np.asarray(Wq, dtype=np.float32)
    Wk = np.asarray(Wk, dtype=np.float32)
    Wv = np.asarray(Wv, dtype=np.float32)
    Wo = np.asarray(Wo, dtype=np.float32)
    bq = np.asarray(bq, dtype=np.float32)
    bk = np.asarray(bk, dtype=np.float32)
    bv = np.asarray(bv, dtype=np.float32)
    bo = np.asarray(bo, dtype=np.float32)

    nc = _get_nc()
    xt_b = [np.ascontiguousarray(
        _tile16(_bf16(x[b].T)).reshape(128, 16, 4, 512).transpose(0, 2, 1, 3))
        for b in range(B)]
    in_maps = []
    for c in range(NCORES):
        b = c // 4
        q = c % 4
        hs = q * HPC * DH          # column start in Wq / row start in Wo
        kv = q // 2
        bq_m = np.ascontiguousarray(
            bq[hs:hs + HPC * DH].reshape(HPC, DH).T)          # [128, 4]
        bk_m = np.ascontiguousarray(
            bk[kv * DH:(kv + 1) * DH].reshape(DH, 1))         # [128, 1]
        bv_m = np.ascontiguousarray(
            bv[kv * DH:(kv + 1) * DH].reshape(DH, 1))         # [128, 1]
        in_maps.append({
            "xt": xt_b[b],
            "wq": np.ascontiguousarray(
                _tile16(_bf16(Wq[:, hs:hs + HPC * DH]))
                .reshape(128, 16, HPC, DH).transpose(0, 2, 1, 3)),
            "wk": _tile16(_bf16(Wk[:, kv * DH:(kv + 1) * DH])),
            "wv": _tile16(_bf16(Wv[:, kv * DH:(kv + 1) * DH])),
            "wo": np.ascontiguousarray(
                _bf16(Wo[hs:hs + HPC * DH, :]).reshape(HPC, 128, D)
                .transpose(1, 0, 2)),
            "bqm": bq_m,
            "bkm": bk_m,
            "bvm": bv_m,
        })

    res = run_bass_kernel_spmd(nc, in_maps, list(range(NCORES)),
                               **kw.get("_run_kwargs", {}))
    if kw.get("_return_res"):
        return res
    parts = [np.asarray(res.results[c]["part"]).astype(np.float32)
             for c in range(NCORES)]
    out = np.empty((B, T, D), dtype=np.float32)
    for b in range(B):
        acc = parts[4 * b]
        for q in range(1, 4):
            acc += parts[4 * b + q]
        out[b] = acc + bo[None, :]
    return out


# revision 21
# speedup vs baseline: 1.1406x; 1.0007x over previous
"""GQA kernel for Trainium2, 8 NeuronCores.

Problem: B=2, T=2048, D=2048, 16 query heads / 2 KV heads, d_head=128, causal.

Sharding: core c -> batch b = c//4, head-quarter q = c%4 (query heads
4q..4q+3, kv head q//2). Each core computes its 4 heads' attention and a
partial output projection (its Wo rows); host sums the 4 partials per batch
and adds bo.

Host marshalling: weights and x are pre-cast to bf16 and pre-tiled so each
logical input lands with one multi-dim DMA per priority chunk:
  xt [128, 16, 2048] = x[b].T tiled as (p, kb, t)
  wq [128, 16, 512], wk/wv [128, 16, 128]  (p, kb, cols)
  wo [128, 4, 2048]                        (p=dh, h, n)
Inputs stream on FOUR queues (sync/gpsimd/vector/scalar) in approximate
consumption order (KB blocks consumed 0..15): the early-engine queues
(vector/scalar) only carry startup chunks, before their first compute.

PE warm-up: the HAM clock gate keeps the PE at 1.2 GHz until ~3.4us of
sustained activity; dummy matmuls bridge the initial DMA wait so real
matmuls start at 2.4 GHz.

On-core dataflow (bf16 matmuls, fp32 PSUM):
per 512-wide t-slice j: K/V projections into one [128,2,512] PSUM pair
tile (ACT epilogues w/ bias); V PE-transposed to natural layout; Q heads
pairwise in PSUM pair tiles.  Attention per head h over tk blocks: S^T
computed per tk-block PAIR into a [128,2,512] PSUM pair tile; ONE fused
exp per pair (ACT reads both banks in one ACTIVATE); causal diagonal
blocks column-trimmed with all four S matmuls issued before their
exp/mask/PV chains (PE never sits in the chain); 128x128 affine_select
masks on gpsimd only for the true diagonal; PV accumulated into otps;
softmax denominator: bf16 pair-tree + fp32 racc on DVE, bf16 cast, one
ones-matmul partition reduction per (h,j); normalize (reciprocal+mul) on
DVE.  Per-head normalize is deferred into the next head's stream.
Output projection for slice j is cut into 4-matmul units used as PE
filler inside slice j+1's attention rounds; a few units are RESERVED to
cover the final head's normalize chain; slice-3 outproj uses freed
PSUM pair tiles and one [128,2048] DMA per row-block.
"""

import numpy as np
import ml_dtypes
from contextlib import ExitStack

import concourse.bass as bass
from concourse import bacc
import concourse.mybir as mybir
import concourse.tile as tile
from concourse.bass_utils import run_bass_kernel_spmd
from concourse.masks import make_identity
from concourse import bass_isa

F32 = mybir.dt.float32
F32R = mybir.dt.float32r
BF16 = mybir.dt.bfloat16

D = 2048
T = 2048
DH = 128
B = 2
HPC = 4            # query heads per core
NCORES = 8
SCALE = 1.0 / float(np.sqrt(128.0))
N_WARM = 9         # PE warm-up dummy matmuls (512-col)
ROWSUM_GPSIMD = False  # denominator partition-reduce on gpsimd vs PE matmul

_CACHE = {}


def _build_nc():
    nc = bacc.Bacc("TRN2", target_bir_lowering=False, debug=False,
                   num_devices=NCORES)

    xt = nc.dram_tensor("xt", [128, 4, 16, 512], BF16, kind="ExternalInput")
    wq = nc.dram_tensor("wq", [128, HPC, 16, DH], BF16, kind="ExternalInput")
    wk = nc.dram_tensor("wk", [128, 16, DH], BF16, kind="ExternalInput")
    wv = nc.dram_tensor("wv", [128, 16, DH], BF16, kind="ExternalInput")
    wo = nc.dram_tensor("wo", [128, HPC, D], BF16, kind="ExternalInput")
    bqm = nc.dram_tensor("bqm", [DH, HPC], F32, kind="ExternalInput")
    bkm = nc.dram_tensor("bkm", [DH, 1], F32, kind="ExternalInput")
    bvm = nc.dram_tensor("bvm", [DH, 1], F32, kind="ExternalInput")
    part = nc.dram_tensor("part", [T, D], BF16, kind="ExternalOutput")

    with ExitStack() as ctx:
        tc = ctx.enter_context(tile.TileContext(nc))
        persist = ctx.enter_context(tc.tile_pool(name="persist", bufs=1))
        work = ctx.enter_context(tc.tile_pool(name="work", bufs=2))
        psum = ctx.enter_context(tc.tile_pool(name="psum", bufs=2, space="PSUM"))

        # ---- constants ----
        ones32 = persist.tile([128, 128], BF16, tag="ones32", name="ones32")
        nc.vector.memset(ones32, 1.0)
        scr512 = persist.tile([128, 512], BF16, tag="scr512", name="scr512")
        nc.vector.memset(scr512, 0.0)
        ident = persist.tile([128, 128], BF16, tag="ident", name="ident")
        make_identity(nc, ident)
        bq_sb = persist.tile([DH, HPC], F32, tag="bq", name="bq_sb")
        bk_sb = persist.tile([DH, 1], F32, tag="bk", name="bk_sb")
        bv_sb = persist.tile([DH, 1], F32, tag="bv", name="bv_sb")

        # ---- persistent input tiles ----
        x_all = persist.tile([128, 4, 16, 512], BF16, tag="x_all", name="x_all")
        wq_sb = persist.tile([128, HPC, 16, DH], BF16, tag="wq", name="wq_sb")
        wk_sb = persist.tile([128, 16, DH], BF16, tag="wk", name="wk_sb")
        wv_sb = persist.tile([128, 16, DH], BF16, tag="wv", name="wv_sb")
        wo_sb = persist.tile([128, HPC, D], BF16, tag="wo", name="wo_sb")

        # ---- input DMAs ----
        # Single fast queue (gpsimd) in exact consumption order; all DMAs
        # are contiguous on both sides (x is slice-major, wq head-major).
        # Only wo (needed ~70us later) rides the slow sync queue.
        nc.gpsimd.dma_start(out=bq_sb, in_=bqm[:, :])
        nc.gpsimd.dma_start(out=bk_sb, in_=bkm[:, :])
        nc.gpsimd.dma_start(out=bv_sb, in_=bvm[:, :])
        nc.gpsimd.dma_start(out=wk_sb, in_=wk[:, :, :])
        nc.gpsimd.dma_start(out=wv_sb, in_=wv[:, :, :])
        for duo in range(6):
            a = slice(2 * duo, 2 * duo + 2)
            nc.gpsimd.dma_start(out=x_all[:, 0, a, :], in_=xt[:, 0, a, :])
        nc.gpsimd.dma_start(out=wq_sb[:, 0], in_=wq[:, 0])
        nc.gpsimd.dma_start(out=x_all[:, 0, 12:14, :], in_=xt[:, 0, 12:14, :])
        nc.gpsimd.dma_start(out=x_all[:, 0, 14:16, :], in_=xt[:, 0, 14:16, :])
        for h in range(1, 4):
            nc.gpsimd.dma_start(out=wq_sb[:, h], in_=wq[:, h])
        for js in range(1, 4):
            nc.gpsimd.dma_start(out=x_all[:, js, 0:8, :], in_=xt[:, js, 0:8, :])
            nc.gpsimd.dma_start(out=x_all[:, js, 8:16, :],
                                in_=xt[:, js, 8:16, :])
        nc.sync.dma_start(out=wo_sb[:, 0:2, :], in_=wo[:, 0:2, :])
        nc.sync.dma_start(out=wo_sb[:, 2:4, :], in_=wo[:, 2:4, :])

        # warm the ACT exp table-set (~2.7us) during the initial DMA wait
        warm = persist.tile([128, 1], F32, tag="warm", name="warm")
        nc.scalar.activation(out=warm, in_=ident[:, 0:1],
                             func=mybir.ActivationFunctionType.Exp)

        # ---- PE warm-up dummies: keep the HAM clock gate open ----
        wps = psum.tile([128, 2, 512], F32, tag="sps", bufs=2, name="wps")
        for i in range(N_WARM):
            nc.tensor.matmul(out=wps[:, i % 2, :], lhsT=ones32, rhs=scr512,
                             start=True, stop=True)

        # ---- persistent activations ----
        kT = persist.tile([128, T], BF16, tag="kT", name="kT")
        v_sb = [persist.tile([128, DH], BF16, tag=f"v{t}", name=f"v{t}")
                for t in range(16)]
        # per-slice q and o (o double-buffered: outproj(j) runs during j+1)
        qT4 = [persist.tile([128, 4, 512], BF16, tag=f"qT{h}", name=f"qT{h}")
               for h in range(HPC)]
        oT = [[persist.tile([128, 512], BF16, tag=f"oT{d}_{h}",
                            name=f"oT{d}_{h}")
               for h in range(HPC)] for d in range(2)]

        # ---------- filler machinery ----------
        fillers = []
        pending_fin = [None]

        def run_pending():
            if pending_fin[0] is not None:
                f = pending_fin[0]
                pending_fin[0] = None
                f()

        def pop_fillers(k, budget=None):
            n = min(k, len(fillers))
            if budget is not None:
                n = min(n, budget[0])
                budget[0] -= n
            if n:
                fillers.pop(0)()
            run_pending()
            for _ in range(n - 1):
                fillers.pop(0)()

        def drain_fillers():
            if fillers:
                fillers.pop(0)()
            run_pending()
            while fillers:
                fillers.pop(0)()

        # ---------- projection helpers ----------
        def qproj_quarter(j, h, qps, kq):
            def emit():
                for kb in range(4 * kq, 4 * kq + 4):
                    nc.tensor.matmul(out=qps,
                                     lhsT=wq_sb[:, h, kb, :],
                                     rhs=x_all[:, j, kb, :],
                                     start=(kb == 0), stop=(kb == 15))
                if kq == 3:
                    nc.scalar.activation(out=qT4[h][:, j, :], in_=qps,
                                         func=mybir.ActivationFunctionType.Identity,
                                         bias=bq_sb[:, h:h + 1], scale=1.0)
            return emit

        def emit_qproj_pair(j, h0, h1):
            qpp = psum.tile([128, 2, 512], F32, tag="sps", bufs=2,
                            name=f"qpp{j}_{h0}")
            for pl, h in ((0, h0), (1, h1)):
                for kq in range(4):
                    qproj_quarter(j, h, qpp[:, pl, :], kq)()

        def vtrans(j, vplane):
            vt_sb = work.tile([128, 512], BF16, tag="vt", bufs=2,
                              name=f"vt{j}")
            nc.scalar.activation(out=vt_sb, in_=vplane,
                                 func=mybir.ActivationFunctionType.Identity,
                                 bias=bv_sb[:, 0:1], scale=1.0)
            vtp = psum.tile([128, 512], BF16, tag="op", bufs=2, name=f"vtp{j}")
            for sub in range(4):
                nc.tensor.transpose(vtp[:, sub * 128:(sub + 1) * 128],
                                    vt_sb[:, sub * 128:(sub + 1) * 128],
                                    ident)
            for sub in range(4):
                nc.vector.tensor_copy(out=v_sb[4 * j + sub],
                                      in_=vtp[:, sub * 128:(sub + 1) * 128])

        def emit_proj_slice0():
            """K/V projections for slice 0, matmuls interleaved at kb
            granularity in DMA arrival order, padded with dummy matmuls to
            keep the PE clock warm while x streams in."""
            kvp = psum.tile([128, 2, 512], F32, tag="sps", bufs=2, name="kvp0")
            q01 = psum.tile([128, 2, 512], F32, tag="sps", bufs=2, name="q01e")
            for kb in range(16):
                st, sp = (kb == 0), (kb == 15)
                nc.tensor.matmul(out=kvp[:, 0, :], lhsT=wk_sb[:, kb, :],
                                 rhs=x_all[:, 0, kb, :], start=st, stop=sp)
                nc.tensor.matmul(out=kvp[:, 1, :], lhsT=wv_sb[:, kb, :],
                                 rhs=x_all[:, 0, kb, :], start=st, stop=sp)
                if kb > 1:
                    nc.tensor.matmul(out=wps[:, kb % 2, :], lhsT=ones32,
                                     rhs=scr512, start=True, stop=True)
                if kb > 3:
                    nc.tensor.matmul(out=wps[:, (kb + 1) % 2, :], lhsT=ones32,
                                     rhs=scr512, start=True, stop=True)
                # interleave Q(0,h0) quarters over already-arrived kb blocks
                if kb == 11:
                    qproj_quarter(0, 0, q01[:, 0, :], 0)()
                elif kb == 13:
                    qproj_quarter(0, 0, q01[:, 0, :], 1)()
                elif kb == 15:
                    qproj_quarter(0, 0, q01[:, 0, :], 2)()
            qproj_quarter(0, 0, q01[:, 0, :], 3)()
            nc.scalar.activation(out=kT[:, 0:512], in_=kvp[:, 0, :],
                                 func=mybir.ActivationFunctionType.Identity,
                                 bias=bk_sb[:, 0:1], scale=1.0)
            vtrans(0, kvp[:, 1, :])
            for kq in range(4):
                qproj_quarter(0, 1, q01[:, 1, :], kq)()

        def emit_kvproj(j):
            sl = slice(j * 512, (j + 1) * 512)
            kvp = psum.tile([128, 2, 512], F32, tag="sps", bufs=2,
                            name=f"kvp{j}")
            for kb in range(16):
                nc.tensor.matmul(out=kvp[:, 0, :], lhsT=wk_sb[:, kb, :],
                                 rhs=x_all[:, j, kb, :],
                                 start=(kb == 0), stop=(kb == 15))
            for kb in range(16):
                nc.tensor.matmul(out=kvp[:, 1, :], lhsT=wv_sb[:, kb, :],
                                 rhs=x_all[:, j, kb, :],
                                 start=(kb == 0), stop=(kb == 15))
            nc.scalar.activation(out=kT[:, sl], in_=kvp[:, 0, :],
                                 func=mybir.ActivationFunctionType.Identity,
                                 bias=bk_sb[:, 0:1], scale=1.0)
            vtrans(j, kvp[:, 1, :])

        # ---------- output projection units (filler fodder) ----------
        _ostg = {}

        def outproj_unit(jj, tt, n, heads, key):
            # matmuls (contract over `heads`) + staging copy + DMA per tt
            def emit():
                if key not in _ostg:
                    _ostg[key] = work.tile([128, 4, 512], BF16, tag="ostg",
                                           bufs=2, name=f"ostg{key}")
                ostg = _ostg[key]
                ops = psum.tile([128, 512], F32, tag="op", bufs=2,
                                name=f"ops{key}_{n}")
                sub = tt % 4
                for i, h in enumerate(heads):
                    nc.tensor.matmul(
                        out=ops,
                        lhsT=oT[jj % 2][h][:, sub * 128:(sub + 1) * 128],
                        rhs=wo_sb[:, h, n * 512:(n + 1) * 512],
                        start=(i == 0), stop=(i == len(heads) - 1))
                if n % 2 == 0:
                    nc.vector.tensor_copy(out=ostg[:, n, :], in_=ops)
                else:
                    nc.scalar.copy(out=ostg[:, n, :], in_=ops)
                if n == 3:
                    nc.gpsimd.dma_start(
                        out=part[tt * 128:(tt + 1) * 128, :], in_=ostg)
            return emit

        def queue_outproj(j):
            for sub in range(4):
                tt = 4 * j + sub
                for n in range(4):
                    fillers.append(
                        outproj_unit(j, tt, n, heads=(0, 1, 2, 3), key=tt))

        def tail_outproj():
            """Slice-3 outproj on freed attention PSUM pair tiles; one
            [128,2048] DMA per row-block."""
            for sub in range(4):
                tt = 12 + sub
                ostg = work.tile([128, 4, 512], BF16, tag="ostg", bufs=2,
                                 name=f"ostgT{tt}")
                for half in range(2):
                    opp = psum.tile([128, 2, 512], F32, tag="sps", bufs=2,
                                    name=f"oppT{tt}_{half}")
                    for pl in range(2):
                        n = 2 * half + pl
                        for i, h in enumerate((0, 1, 2, 3)):
                            nc.tensor.matmul(
                                out=opp[:, pl, :],
                                lhsT=oT[1][h][:, (tt % 4) * 128:
                                              (tt % 4 + 1) * 128],
                                rhs=wo_sb[:, h, n * 512:(n + 1) * 512],
                                start=(i == 0), stop=(i == 3))
                    dst = ostg[:, 2 * half:2 * half + 2, :]
                    if (sub + half) % 2 == 0:
                        nc.vector.tensor_copy(out=dst, in_=opp)
                    else:
                        nc.scalar.copy(out=dst, in_=opp)
                nc.gpsimd.dma_start(out=part[tt * 128:(tt + 1) * 128, :],
                                    in_=ostg)

        # ---------- attention ----------
        def emit_attention_head(j, h, fill_rate, budget=None):
            """Attention for head h over tq-slice j, tk blocks 0..4j+3.
            S^T per tk-block PAIR into a [128,2,512] PSUM pair tile with a
            single fused exp; the 4 diagonal blocks are column-trimmed and
            their S matmuls issued before the exp/mask/PV chains."""
            otps = psum.tile([128, 512], F32, tag="acc", bufs=2,
                             name=f"otps{h}_{j}")
            racc = work.tile([128, 512], F32, tag="racc", bufs=2,
                             name=f"racc{h}_{j}")

            def pv_mm(tkb, pt_ap, o_off, start, stop):
                nc.tensor.matmul(out=otps[:, o_off:512], lhsT=v_sb[tkb],
                                 rhs=pt_ap, start=start, stop=stop,
                                 skip_group_check=True)

            pps = []
            racc_init = [False]

            def racc_accum(ap):
                if not racc_init[0]:
                    nc.vector.tensor_copy(out=racc, in_=ap)
                    racc_init[0] = True
                else:
                    nc.vector.tensor_add(out=racc, in0=racc, in1=ap)

            # --- non-diagonal block pairs ---
            for p in range(2 * j):
                spair = psum.tile([128, 2, 512], F32, tag="sps", bufs=2,
                                  name=f"sp{h}_{j}_{p}")
                for q in range(2):
                    nc.tensor.matmul(
                        out=spair[:, q, :],
                        lhsT=kT[:, (2 * p + q) * 128:(2 * p + q + 1) * 128],
                        rhs=qT4[h][:, j, :], start=True, stop=True)
                ptp = work.tile([128, 2, 512], BF16, tag="pt", bufs=6,
                                name=f"pt{h}_{j}_{p}")
                nc.scalar.activation(out=ptp, in_=spair,
                                     func=mybir.ActivationFunctionType.Exp,
                                     scale=SCALE)
                pv_mm(2 * p, ptp[:, 0, :], 0, start=(p == 0), stop=False)
                pv_mm(2 * p + 1, ptp[:, 1, :], 0, start=False, stop=False)
                pp = work.tile([128, 512], BF16, tag="ppair", bufs=4,
                               name=f"pp{h}_{j}_{p}")
                nc.vector.tensor_add(out=pp, in0=ptp[:, 0, :],
                                     in1=ptp[:, 1, :])
                pps.append(pp)
                if len(pps) == 2:
                    qs = work.tile([128, 512], BF16, tag="qsum", bufs=2,
                                   name=f"qs{h}_{j}_{p}")
                    nc.vector.tensor_add(out=qs, in0=pps[0], in1=pps[1])
                    pps.clear()
                    racc_accum(qs)
                pop_fillers(fill_rate, budget)
            if pps:
                racc_accum(pps[0])
                pps.clear()

            # --- diagonal blocks r=0..3, column-trimmed ---
            # All four S matmuls first (two pair tiles), then fillers, then
            # the exp/mask/PV chains so the PE is never inside the chain.
            base = 4 * j
            dp = [psum.tile([128, 2, 512], F32, tag="sps", bufs=2,
                            name=f"dp{h}_{j}_{i}") for i in range(2)]
            for r in range(4):
                w = 512 - 128 * r
                nc.tensor.matmul(
                    out=dp[r // 2][:, r % 2, 0:w],
                    lhsT=kT[:, (base + r) * 128:(base + r + 1) * 128],
                    rhs=qT4[h][:, j, 128 * r:512],
                    start=True, stop=True)
            pop_fillers(fill_rate, budget)
            ptd = [work.tile([128, 2, 512], BF16, tag="pt", bufs=6,
                             name=f"ptd{h}_{j}_{i}") for i in range(2)]
            for r in range(4):
                w = 512 - 128 * r
                nc.scalar.activation(
                    out=ptd[r // 2][:, r % 2, 0:w],
                    in_=dp[r // 2][:, r % 2, 0:w],
                    func=mybir.ActivationFunctionType.Exp, scale=SCALE)
            for r in range(4):
                nc.gpsimd.affine_select(
                    out=ptd[r // 2][:, r % 2, 0:128],
                    in_=ptd[r // 2][:, r % 2, 0:128],
                    compare_op=mybir.AluOpType.is_ge,
                    fill=0.0, base=0,
                    pattern=[[1, 128]], channel_multiplier=-1)
            for r in range(4):
                w = 512 - 128 * r
                w_off = 128 * r
                pv_mm(base + r, ptd[r // 2][:, r % 2, 0:w], w_off,
                      start=(j == 0 and r == 0), stop=(r == 3))
                if r == 0:
                    racc_accum(ptd[0][:, 0, :])
                else:
                    nc.vector.tensor_add(out=racc[:, w_off:512],
                                         in0=racc[:, w_off:512],
                                         in1=ptd[r // 2][:, r % 2, 0:w])
                if r % 2 == 1:
                    pop_fillers(fill_rate, budget)

            # --- denominator: partition reduction ---
            if ROWSUM_GPSIMD:
                def finish():
                    rs = work.tile([128, 512], F32, tag="rs", bufs=2,
                                   name=f"rs{h}_{j}")
                    nc.gpsimd.partition_all_reduce(
                        rs, racc, 128, bass_isa.ReduceOp.add)
                    rinv = work.tile([128, 512], F32, tag="rinv", bufs=2,
                                     name=f"rinv{h}_{j}")
                    nc.vector.reciprocal_approx_fast(rinv, rs)
                    nc.vector.tensor_mul(out=oT[j % 2][h], in0=otps,
                                         in1=rinv)
                return finish
            racc16 = work.tile([128, 512], BF16, tag="racc16", bufs=2,
                               name=f"racc16{h}_{j}")
            nc.vector.tensor_copy(out=racc16, in_=racc)

            def finish():
                rsb = psum.tile([128, 512], F32, tag="op", bufs=2,
                                name=f"rsb{h}_{j}")
                nc.tensor.matmul(out=rsb, lhsT=ones32, rhs=racc16,
                                 start=True, stop=True)
                rinv = work.tile([128, 512], F32, tag="rinv", bufs=2,
                                 name=f"rinv{h}_{j}")
                nc.vector.reciprocal_approx_fast(rinv, rsb)
                nc.vector.tensor_mul(out=oT[j % 2][h], in0=otps, in1=rinv)
            return finish

        # ---------- main schedule ----------
        # Phase A: stream-paced projections (KV+Q per slice, in x arrival
        # order) with attention(0) filling the xs1 arrival gap.
        emit_proj_slice0()
        emit_qproj_pair(0, 2, 3)
        pending_fin[0] = emit_attention_head(0, 0, fill_rate=1)
        pending_fin[0] = emit_attention_head(0, 1, fill_rate=1)
        pending_fin[0] = emit_attention_head(0, 2, fill_rate=1)
        fin0 = emit_attention_head(0, 3, fill_rate=1)
        for js in range(1, 4):
            emit_kvproj(js)
            if js == 1:
                fin0()
            emit_qproj_pair(js, 0, 1)
            emit_qproj_pair(js, 2, 3)
        # Phase B: attention(1..3) with outproj fillers.
        fin3 = [None]
        for j in range(1, 4):
            # flush the deferred normalize of head (j-1, 3) BEFORE any
            # outproj(j-1) unit can pop -- those units read oT[(j-1)%2]
            run_pending()
            queue_outproj(j - 1)
            if j < 3:
                pending_fin[0] = emit_attention_head(j, 0, fill_rate=1)
                pending_fin[0] = emit_attention_head(j, 1, fill_rate=1)
                pending_fin[0] = emit_attention_head(j, 2, fill_rate=1)
                pending_fin[0] = emit_attention_head(j, 3, fill_rate=1)
            else:
                pending_fin[0] = emit_attention_head(3, 0, fill_rate=1,
                                                     budget=[5])
                pending_fin[0] = emit_attention_head(3, 1, fill_rate=1,
                                                     budget=[4])
                pending_fin[0] = emit_attention_head(3, 2, fill_rate=1,
                                                     budget=[3])
                fin3[0] = emit_attention_head(3, 3, fill_rate=1, budget=[2])
                run_pending()
                fin3[0]()
                fin3[0] = None
                drain_fillers()
                tail_outproj()
        drain_fillers()

    nc.compile()
    return nc


def _get_nc():
    if "nc" not in _CACHE:
        _CACHE["nc"] = _build_nc()
    return _CACHE["nc"]


def _bf16(a):
    return np.ascontiguousarray(a.astype(ml_dtypes.bfloat16))


def _tile16(a):
    # [2048, C] -> [128, 16, C]   (rows kb*128+p -> [p, kb, :])
    c = a.shape[1]
    return np.ascontiguousarray(
        a.reshape(16, 128, c).transpose(1, 0, 2))


def kernel(x, Wq, bq, Wk, bk, Wv, bv, Wo, bo, **kw):
    x = np.asarray(x, dtype=np.float32)
    Wq = np.asarray(Wq, dtype=np.float32)
    Wk = np.asarray(Wk, dtype=np.float32)
    Wv = np.asarray(Wv, dtype=np.float32)
    Wo = np.asarray(Wo, dtype=np.float32)
    bq = np.asarray(bq, dtype=np.float32)
    bk = np.asarray(bk, dtype=np.float32)
    bv = np.asarray(bv, dtype=np.float32)
    bo = np.asarray(bo, dtype=np.float32)

    nc = _get_nc()
    xt_b = [np.ascontiguousarray(
        _tile16(_bf16(x[b].T)).reshape(128, 16, 4, 512).transpose(0, 2, 1, 3))
        for b in range(B)]
    in_maps = []
    for c in range(NCORES):
        b = c // 4
        q = c % 4
        hs = q * HPC * DH          # column start in Wq / row start in Wo
        kv = q // 2
        bq_m = np.ascontiguousarray(
            bq[hs:hs + HPC * DH].reshape(HPC, DH).T)          # [128, 4]
        bk_m = np.ascontiguousarray(
            bk[kv * DH:(kv + 1) * DH].reshape(DH, 1))         # [128, 1]
        bv_m = np.ascontiguousarray(
            bv[kv * DH:(kv + 1) * DH].reshape(DH, 1))         # [128, 1]
        in_maps.append({
            "xt": xt_b[b],
            "wq": np.ascontiguousarray(
                _tile16(_bf16(Wq[:, hs:hs + HPC * DH]))
                .reshape(128, 16, HPC, DH).transpose(0, 2, 1, 3)),
            "wk": _tile16(_bf16(Wk[:, kv * DH:(kv + 1) * DH])),
            "wv": _tile16(_bf16(Wv[:, kv * DH:(kv + 1) * DH])),
            "wo": np.ascontiguousarray(
                _bf16(Wo[hs:hs + HPC * DH, :]).reshape(HPC, 128, D)
                .transpose(1, 0, 2)),
            "bqm": bq_m,
            "bkm": bk_m,
            "bvm": bv_m,
        })

    res = run_bass_kernel_spmd(nc, in_maps, list(range(NCORES)),
                               **kw.get("_run_kwargs", {}))
    if kw.get("_return_res"):
        return res
    parts = [np.asarray(res.results[c]["part"]).astype(np.float32)
             for c in range(NCORES)]
    out = np.empty((B, T, D), dtype=np.float32)
    for b in range(B):
        acc = parts[4 * b]
        for q in range(1, 4):
            acc += parts[4 * b + q]
        out[b] = acc + bo[None, :]
    return out
